# revision 65
# baseline (speedup 1.0000x reference)
"""Trainium2 Bass kernel for nn_AdaptiveResidualCombinedEncoder.

Pure data-parallel over 8 NeuronCores: batch 2048 -> 256 rows/core.

Key idea: all the soft-shift ops (_shift_last / _shift_channels) have
batch-independent indices & lerp weights, so they are banded linear
operators that the host can precompute:
  - per-branch shift "masks" (band diagonals, gains folded in) applied
    with shifted-AP vector ops,
  - a [128,128] block-diagonal channel-shift matrix for the spikes
    branch applied as bf16 TensorEngine matmuls (two k=64 chunks per
    row so the HBM reads use 2KB descriptors via channel-pair packing),
  - split base/residual weight stacks so relu(base) + scale*res uses
    two accumulated matmul groups (bias via a k=1 ones matmul).
The spikes t-summary falls out of the PSUM->SBUF evacuation for free via
the accum_out feature (per-row running sum on DVE/ACT), and the whole
kernel is HBM-bandwidth-bound (~74MB/core) at ~92% of the practical
packet ceiling.
"""

from contextlib import ExitStack

import numpy as np

import concourse.bass as bass
import concourse.tile as tile
from concourse import bacc, mybir
from concourse.bass_utils import run_bass_kernel_spmd

F32 = mybir.dt.float32
BF16 = mybir.dt.bfloat16
ALU = mybir.AluOpType
ACT = mybir.ActivationFunctionType

N_CORES = 8
B = 2048
BC = B // N_CORES            # 256 rows per core
P = 128                      # partitions / rows per tile
NT = BC // P                 # 2 row-tiles per core
EARS, NFC, T = 2, 64, 256
EC = EARS * NFC              # 128 (ears*channels)
D_DIM, A_DIM, E_DIM, H = 256, 256, 192, 512
OUT_COLS = 3 * H + EC * T    # 34304
RG = 8                       # spikes rows per DMA batch
EPS = 1e-5


# ---------------------------------------------------------------- host math
def _np_gain(p):
    return (1.0 + 0.35 * np.tanh(p.astype(np.float64))).astype(np.float32)


def _shift_weights(d, offsets, max_shift):
    """Per-position (lo, hi, a) of the reference soft shift."""
    base = np.arange(d, dtype=np.float32)
    s = base + np.float32(max_shift) * np.tanh(offsets.astype(np.float32))
    s = np.clip(s, 0.0, np.float32(d - 1)).astype(np.float32)
    lo = np.floor(s).astype(np.int64)
    hi = np.ceil(s).astype(np.int64)
    a = (s - lo.astype(np.float32)).astype(np.float32)
    return lo, hi, a


def _band_masks(widths, offs, max_shifts, gains, max_delta):
    """Band-diagonal masks for concatenated shift blocks.

    Returns [2*max_delta+1, sum(widths)] f32: M[di][c] multiplies
    x[c + (di - max_delta)] accumulated into output position c.
    """
    total = int(np.sum(widths))
    n_d = 2 * max_delta + 1
    M = np.zeros((n_d, total), np.float32)
    c0 = 0
    for w, off, ms, g in zip(widths, offs, max_shifts, gains):
        if off is None:  # pure diagonal (gain only)
            M[max_delta, c0:c0 + w] += g
        else:
            lo, hi, a = _shift_weights(w, off, ms)
            for c in range(w):
                M[lo[c] - c + max_delta, c0 + c] += (1.0 - a[c]) * g[c]
                M[hi[c] - c + max_delta, c0 + c] += a[c] * g[c]
        c0 += w
    return M


def _shift_matrix(d, offsets, max_shift, gain):
    """Dense [d, d]: out[c] = sum_src M[c, src] * x[src], gain folded."""
    lo, hi, a = _shift_weights(d, offsets, max_shift)
    M = np.zeros((d, d), np.float32)
    idx = np.arange(d)
    np.add.at(M, (idx, lo), (1.0 - a) * gain)
    np.add.at(M, (idx, hi), a * gain)
    return M


# consts column layout, shared between build_graph and kernel
_CW = {}
_off = 0
for _name, _w in [
    ("wcat_d", 4 * H), ("wcat_a", 4 * H), ("wcat_e", 4 * H),
    ("m2t", EC), ("ident", P),
]:
    _CW[_name] = (_off, _w)
    _off += _w
CONST_COLS = _off

# mask row layout (one-partition param, broadcast on-chip)
_MW = {"mask_d": (0, 3 * D_DIM), "mask_a": (3 * D_DIM, 3 * A_DIM),
       "mask_e": (3 * D_DIM + 3 * A_DIM, 5 * E_DIM)}
MASK_COLS = 3 * D_DIM + 3 * A_DIM + 5 * E_DIM  # 2496


# ---------------------------------------------------------------- device IR
def build_graph(debug=False):
    """One SPMD graph, identical on all cores; values arrive via params."""
    nc = bacc.Bacc(None, target_bir_lowering=False)

    dist_e = nc.dram_tensor("distance", [BC, D_DIM], F32, kind="ExternalInput")
    az_e = nc.dram_tensor("azimuth", [BC, A_DIM], F32, kind="ExternalInput")
    elev_e = nc.dram_tensor("elevation", [BC, E_DIM], F32, kind="ExternalInput")
    sp_e = nc.dram_tensor("spikes", [BC, EC, T], F32, kind="ExternalInput")
    consts_e = nc.dram_tensor("consts", [P, CONST_COLS], F32, kind="ExternalInput")
    mask_e = nc.dram_tensor("mask_row", [1, MASK_COLS], F32, kind="ExternalInput")
    bias_e = nc.dram_tensor("bias_cat", [1, 6 * H], F32, kind="ExternalInput")
    m2q_e = nc.dram_tensor("m2q", [P, 4 * EC], F32, kind="ExternalInput")
    out_e = nc.dram_tensor("out", [BC, OUT_COLS], F32, kind="ExternalOutput")
    if debug:
        dbg_ad = nc.dram_tensor("dbg_ad", [P, D_DIM], F32, kind="ExternalOutput")
        dbg_ln = nc.dram_tensor("dbg_ln", [P, D_DIM], F32, kind="ExternalOutput")
        dbg_lhs = nc.dram_tensor("dbg_lhs", [P, 4 * P], F32, kind="ExternalOutput")
        dbg_ps = nc.dram_tensor("dbg_ps", [P, H], F32, kind="ExternalOutput")
        dbg_sta = nc.dram_tensor("dbg_sta", [P, P], F32, kind="ExternalOutput")
        dbg_lns = nc.dram_tensor("dbg_lns", [P, P], F32, kind="ExternalOutput")

    with ExitStack() as ctx:
        tc = ctx.enter_context(tile.TileContext(nc))
        cpool = ctx.enter_context(tc.tile_pool(name="consts", bufs=1))
        inpool = ctx.enter_context(tc.tile_pool(name="inputs", bufs=1))
        sp_in_pool = ctx.enter_context(tc.tile_pool(name="sp_in", bufs=4))
        sp_out_pool = ctx.enter_context(tc.tile_pool(name="sp_out", bufs=4))
        st_pool = ctx.enter_context(tc.tile_pool(name="st", bufs=2))
        work = ctx.enter_context(tc.tile_pool(name="work", bufs=2))
        lhs_pool = ctx.enter_context(tc.tile_pool(name="lhs", bufs=2))
        lat_pool = ctx.enter_context(tc.tile_pool(name="lat", bufs=2))
        stats = ctx.enter_context(tc.tile_pool(name="stats", bufs=4))
        ps_sp = ctx.enter_context(tc.tile_pool(name="ps_sp", bufs=3, space="PSUM"))
        ps_tr = ctx.enter_context(tc.tile_pool(name="ps_tr", bufs=2, space="PSUM"))
        ps_sum = ctx.enter_context(tc.tile_pool(name="ps_sum", bufs=1, space="PSUM"))
        ps_lat = ctx.enter_context(tc.tile_pool(name="ps_lat", bufs=2, space="PSUM"))

        consts = cpool.tile([P, CONST_COLS], F32)
        nc.sync.dma_start(consts[:], consts_e[:])
        biasr = cpool.tile([1, 6 * H], F32)
        nc.sync.dma_start(biasr[:], bias_e[:])
        ones_r = cpool.tile([1, P], F32)
        nc.vector.memset(ones_r[:], 1.0)
        # bf16 spike-shift matrices: four source-channel residue
        # classes (mod 4), host-duplicated across partition quarters
        # (for the k=32 channel-quad matmuls); cast-DMA f32->bf16
        m2q = []
        for c in range(4):
            mt = cpool.tile([P, EC], BF16, tag=f"m2q{c}")
            nc.gpsimd.dma_start(mt[:], m2q_e[:, c * EC:(c + 1) * EC])
            m2q.append(mt)
        eps_t = cpool.tile([P, 1], F32)
        nc.vector.memset(eps_t[:], float(EPS))

        def cslice(name, j=None, w=None):
            o, full = _CW[name]
            if j is not None:
                return consts[:, o + j * w: o + (j + 1) * w]
            return consts[:, o: o + full]

        ident = cslice("ident")

        # broadcast the 1-partition mask row to all 128 partitions via PE
        mrow = cpool.tile([1, MASK_COLS], F32)
        nc.sync.dma_start(mrow[:], mask_e[:])
        masks = cpool.tile([P, MASK_COLS], F32)
        for j in range((MASK_COLS + H - 1) // H):
            c0, c1 = j * H, min((j + 1) * H, MASK_COLS)
            psm = ps_lat.tile([P, H], F32, tag="lat")
            nc.tensor.matmul(psm[:, 0:c1 - c0], ones_r[:], mrow[:, c0:c1],
                             start=True, stop=True)
            nc.vector.tensor_copy(masks[:, c0:c1], psm[:, 0:c1 - c0])

        # whole-core input loads, [P, NT, F] so tile t is [:, t, :]
        distL = inpool.tile([P, NT, D_DIM], F32)
        nc.sync.dma_start(distL[:], dist_e[:].rearrange("(t p) f -> p t f", p=P))
        azL = inpool.tile([P, NT, A_DIM], F32)
        nc.sync.dma_start(azL[:], az_e[:].rearrange("(t p) f -> p t f", p=P))
        elevL = inpool.tile([P, NT, E_DIM], F32)
        nc.sync.dma_start(elevL[:], elev_e[:].rearrange("(t p) f -> p t f", p=P))

        def adapted_from_masks(x, mask_name, width, ndelta):
            """ad[:, c] = sum_d x[:, c + d - md] * M_d[:, c] via shifted APs."""
            md = ndelta // 2
            ad = work.tile([P, width], F32, tag=f"ad_{width}")
            tmp = work.tile([P, width], F32, tag=f"tmp_{width}")
            o, _ = _MW[mask_name]
            mk = lambda j: masks[:, o + j * width: o + (j + 1) * width]
            nc.vector.tensor_tensor(ad[:], x, mk(md), op=ALU.mult)
            for d in range(ndelta):
                sh = d - md  # source offset
                if sh == 0:
                    continue
                if sh < 0:
                    dst, src = slice(-sh, width), slice(0, width + sh)
                else:
                    dst, src = slice(0, width - sh), slice(sh, width)
                nc.vector.tensor_tensor(tmp[:, dst], x[:, src], mk(d)[:, dst],
                                        op=ALU.mult)
                nc.vector.tensor_tensor(ad[:, dst], ad[:, dst], tmp[:, dst],
                                        op=ALU.add)
            return ad

        def layernorm(x_ap, width, tag):
            """Return ln tile [P, width] (SBUF), rows on partitions."""
            st6 = stats.tile([P, 6], F32, tag=f"st6_{tag}")
            nc.vector.bn_stats(st6[:], x_ap)
            mv = stats.tile([P, 2], F32, tag=f"mv_{tag}")
            nc.vector.bn_aggr(mv[:], st6[:])
            std = stats.tile([P, 1], F32, tag=f"std_{tag}")
            nc.scalar.activation(std[:], mv[:, 1:2], ACT.Sqrt, bias=eps_t[:])
            rstd = stats.tile([P, 1], F32, tag=f"rstd_{tag}")
            nc.vector.reciprocal(rstd[:], std[:])
            ln = work.tile([P, width], F32, tag=f"ln_{tag}")
            nc.vector.tensor_scalar(ln[:], x_ap, mv[:, 0:1], rstd[:],
                                    op0=ALU.subtract, op1=ALU.mult)
            return ln

        def transpose_into(dst_ap, src_ap, ncols):
            """dst[f, r] = src[r, f]; src [P, ncols] -> dst [ncols, P]."""
            tr = ps_tr.tile([P, P], F32, tag="tr")
            nc.tensor.transpose(tr[:ncols, :], src_ap, ident)
            nc.vector.tensor_copy(dst_ap, tr[:ncols, :])

        def mm_group(ps, chunks, bias_off, wname):
            """PSUM group: ones@bias then each (lhs_ap, w_ap) chunk."""
            nc.tensor.matmul(ps[:], ones_r[:],
                             biasr[:, bias_off:bias_off + H],
                             start=True, stop=False)
            for i, (lhs_ap, w_ap) in enumerate(chunks):
                nc.tensor.matmul(ps[:], lhs_ap, w_ap,
                                 start=False, stop=(i == len(chunks) - 1))

        def branch_epilogue(ps_base, ps_res, boff, t):
            """lat = relu(relu(base) + res) -> out."""
            rb = lat_pool.tile([P, H], F32, tag="relu_base")
            nc.scalar.activation(rb[:], ps_base[:], ACT.Relu)
            pre = lat_pool.tile([P, H], F32, tag="lat_pre")
            nc.vector.scalar_tensor_tensor(pre[:], ps_res[:], 1.0, rb[:],
                                           op0=ALU.mult, op1=ALU.add)
            lat = lat_pool.tile([P, H], F32, tag="lat_sb")
            nc.scalar.activation(lat[:], pre[:], ACT.Relu)
            nc.sync.dma_start(out_e[t * P:(t + 1) * P, boff:boff + H], lat[:])

        def spikes_batch(STa, r0, g, in_chunks=1, out_chunks=1):
            b0 = r0 + g * RG
            # channel-quad layout: partition quarter b holds chans
            # (4q..4q+3) of rows r%4==b -> 4KB HBM read descriptors;
            # the four rows of a group run in disjoint PE row-groups
            spi = sp_in_pool.tile([P, RG // 4, 4 * T], BF16, tag="spi")
            src = sp_e[b0:b0 + RG].rearrange(
                "(r4 four) (q qd) t -> four q r4 (qd t)", four=4, qd=4)
            cw = (RG // 4) // max(in_chunks // 2, 1)
            for s0 in range(0, RG // 4, cw):
                for par in range(4):
                    nc.gpsimd.dma_start(
                        spi[par * 32:(par + 1) * 32, s0:s0 + cw, :],
                        src[par][:, s0:s0 + cw, :])
            spo = sp_out_pool.tile([P, RG, T], F32, tag="spo")
            for r in range(RG):
                r4, base = r // 4, (r % 4) * 32
                psb = ps_sp.tile([P, T], F32, tag="ps_sp")
                tp = (base, 0) if base == 96 else None
                for c in range(4):
                    nc.tensor.matmul(
                        psb[:], m2q[c][base:base + 32, :],
                        spi[base:base + 32, r4, c * T:(c + 1) * T],
                        start=(c == 0), stop=(c == 3), tile_position=tp)
                c0 = g * RG + r
                if (r // 2) % 2 == 0:   # DVE: copy + per-row sum
                    nc.vector.tensor_scalar(
                        spo[:, r, :], psb[:], 1.0, None,
                        op0=ALU.mult, op1=ALU.add,
                        accum_out=STa[:, c0:c0 + 1])
                else:                   # ACT: copy + per-row sum
                    nc.scalar.activation(
                        spo[:, r, :], psb[:], ACT.Copy,
                        accum_out=STa[:, c0:c0 + 1])
            ow = RG // out_chunks
            for c in range(out_chunks):
                s0 = c * ow
                nc.sync.dma_start(
                    out_e[b0 + s0:b0 + s0 + ow, 3 * H:].rearrange(
                        "r (p t) -> p r t", p=P),
                    spo[:, s0:s0 + ow, :])

        def branch_d(t):
            # -------- distance branch
            xd = distL[:, t, :]
            ad = adapted_from_masks(xd, "mask_d", D_DIM, 3)
            ln_d = layernorm(ad[:], D_DIM, "d")
            lhs_d = lhs_pool.tile([P, 4, P], F32, tag="lhs_d")
            transpose_into(lhs_d[:, 0, :], xd[:, 0:P], P)
            transpose_into(lhs_d[:, 1, :], xd[:, P:2 * P], P)
            transpose_into(lhs_d[:, 2, :], ln_d[:, 0:P], P)
            transpose_into(lhs_d[:, 3, :], ln_d[:, P:2 * P], P)
            if debug and t == 0:
                nc.sync.dma_start(dbg_ad[:], ad[:])
                nc.sync.dma_start(dbg_ln[:], ln_d[:])
                nc.sync.dma_start(
                    dbg_lhs[:], lhs_d[:].rearrange("p a b -> p (a b)"))
            ps_b = ps_lat.tile([P, H], F32, tag="lat")
            mm_group(ps_b, [(lhs_d[:, j, :], cslice("wcat_d", j, H))
                            for j in (0, 1)], 0 * H, "wcat_d")
            ps_r = ps_lat.tile([P, H], F32, tag="lat")
            mm_group(ps_r, [(lhs_d[:, j, :], cslice("wcat_d", j, H))
                            for j in (2, 3)], 3 * H, "wcat_d")
            branch_epilogue(ps_b, ps_r, 0 * H, t)

        def branch_a(t):
            xa = azL[:, t, :]
            aa = adapted_from_masks(xa, "mask_a", A_DIM, 3)
            ln_a = layernorm(aa[:], A_DIM, "a")
            lhs_a = lhs_pool.tile([P, 4, P], F32, tag="lhs_a")
            transpose_into(lhs_a[:, 0, :], xa[:, 0:P], P)
            transpose_into(lhs_a[:, 1, :], xa[:, P:2 * P], P)
            transpose_into(lhs_a[:, 2, :], ln_a[:, 0:P], P)
            transpose_into(lhs_a[:, 3, :], ln_a[:, P:2 * P], P)
            ps_b = ps_lat.tile([P, H], F32, tag="lat")
            mm_group(ps_b, [(lhs_a[:, j, :], cslice("wcat_a", j, H))
                            for j in (0, 1)], 1 * H, "wcat_a")
            ps_r = ps_lat.tile([P, H], F32, tag="lat")
            mm_group(ps_r, [(lhs_a[:, j, :], cslice("wcat_a", j, H))
                            for j in (2, 3)], 4 * H, "wcat_a")
            branch_epilogue(ps_b, ps_r, 1 * H, t)

        def branch_e(STa, t):
            # -------- elevation (+ spike-summary residual) branch
            xe = elevL[:, t, :]
            ae = adapted_from_masks(xe, "mask_e", E_DIM, 5)
            ln_e = layernorm(ae[:], E_DIM, "e")

            # summary rows: transpose STa -> [rows, EC feats], LN from PSUM
            trs = ps_sum.tile([P, P], F32, tag="trs")
            nc.tensor.transpose(trs[:], STa[:], ident)
            ln_s = layernorm(trs[:], EC, "s")
            if debug and t == 0:
                nc.sync.dma_start(dbg_sta[:], STa[:])
                nc.sync.dma_start(dbg_lns[:], ln_s[:])

            lhs_e = lhs_pool.tile([P, 4, P], F32, tag="lhs_e")
            transpose_into(lhs_e[:, 0, :], xe[:, 0:P], P)
            # chunk 1 mixes elev[128:192] and ln_e[0:64]: concat in free
            # dim first, then one full 128-wide transpose
            ecat = work.tile([P, P], F32, tag="ecat")
            nc.vector.tensor_copy(ecat[:, 0:64], xe[:, P:E_DIM])
            nc.vector.tensor_copy(ecat[:, 64:P], ln_e[:, 0:64])
            transpose_into(lhs_e[:, 1, :], ecat[:], P)
            transpose_into(lhs_e[:, 2, :], ln_e[:, 64:E_DIM], P)
            transpose_into(lhs_e[:, 3, :], ln_s[:], P)
            ps_b = ps_lat.tile([P, H], F32, tag="lat")
            mm_group(ps_b, [
                (lhs_e[:, 0, :], cslice("wcat_e", 0, H)),
                (lhs_e[0:64, 1, :], cslice("wcat_e", 1, H)[0:64, :]),
            ], 2 * H, "wcat_e")
            ps_r = ps_lat.tile([P, H], F32, tag="lat")
            mm_group(ps_r, [
                (lhs_e[64:P, 1, :], cslice("wcat_e", 1, H)[64:P, :]),
                (lhs_e[:, 2, :], cslice("wcat_e", 2, H)),
                (lhs_e[:, 3, :], cslice("wcat_e", 3, H)),
            ], 5 * H, "wcat_e")
            branch_epilogue(ps_b, ps_r, 2 * H, t)

        NB = P // RG
        for t in range(NT):
            r0 = t * P
            STa = st_pool.tile([P, P], F32, tag="STa")
            for g in range(NB):
                # fine-grained first load (pipeline fill) and final
                # stores (drain) at the kernel boundaries
                ic = 4 if (t == 0 and g == 0) else 1
                oc = 4 if (t == NT - 1 and g == NB - 1) else 1
                spikes_batch(STa, r0, g, in_chunks=ic, out_chunks=oc)
            branch_d(t)
            branch_a(t)
            branch_e(STa, t)

    return nc


_GRAPH_CACHE = {}


def get_graph():
    if "nc" not in _GRAPH_CACHE:
        nc = build_graph()
        nc.finalize()
        _GRAPH_CACHE["nc"] = nc
    return _GRAPH_CACHE["nc"]


def host_prep(inputs):
    """Shard + precompute the derived constant tensors -> in_maps."""
    f = {k: np.asarray(v) for k, v in inputs.items()}
    dh, ah = D_DIM // 2, A_DIM // 2

    mask_d = _band_masks(
        [dh, dh], [f["d_left_off"], f["d_right_off"]], [0.75, 0.75],
        [_np_gain(f["d_left_g"]), _np_gain(f["d_right_g"])], 1)
    mask_a = _band_masks(
        [ah, ah], [f["az_itd_off"], None], [0.75, None],
        [_np_gain(f["az_itd_g"]), _np_gain(f["az_ild_g"])], 1)
    mask_e = _band_masks(
        [NFC, NFC, NFC],
        [f["el_norm_off"], f["el_notch_off"], f["el_slope_off"]],
        [1.5, 1.5, 1.5],
        [_np_gain(f["el_norm_g"]), _np_gain(f["el_notch_g"]),
         _np_gain(f["el_slope_g"])], 2)

    def sigmoid(x):
        return np.float32(1.0 / (1.0 + np.exp(-np.float64(x))))

    d_scale = np.float32(0.35) * sigmoid(f["dist_gain"])
    a_scale = np.float32(0.35) * sigmoid(f["az_gain"])
    e_scale = np.float32(0.35) * sigmoid(f["el_gain"])

    wcat_d = np.vstack([f["bWd"], d_scale * f["Wd"]]).astype(np.float32)
    wcat_a = np.vstack([f["bWa"], a_scale * f["Wa"]]).astype(np.float32)
    wcat_e = np.vstack([f["bWe"], e_scale * f["We"],
                        np.float32(0.25) * e_scale * f["Wsp"]]
                       ).astype(np.float32)
    bias_cat = np.concatenate([
        f["bbd"], f["bba"], f["bbe"],
        d_scale * f["bd"], a_scale * f["ba"],
        e_scale * f["be"] + np.float32(0.25) * e_scale * f["bsp"],
    ]).astype(np.float32)[None, :]

    m_ch = _shift_matrix(NFC, f["spec_off"], 1.5, _np_gain(f["spec_g"]))
    m2 = np.kron(np.eye(EARS, dtype=np.float32), m_ch)   # [EC, EC]
    m2t = np.ascontiguousarray(m2.T)
    # residue-class quarters [32, EC] tiled to [128, EC] each
    m2q_host = np.concatenate(
        [np.tile(m2t[c::4, :], (4, 1)) for c in range(4)], axis=1)

    consts = np.concatenate([
        wcat_d.reshape(4, P, H).transpose(1, 0, 2).reshape(P, 4 * H),
        wcat_a.reshape(4, P, H).transpose(1, 0, 2).reshape(P, 4 * H),
        wcat_e.reshape(4, P, H).transpose(1, 0, 2).reshape(P, 4 * H),
        m2t, np.eye(P, dtype=np.float32),
    ], axis=1).astype(np.float32)
    consts = np.ascontiguousarray(consts)
    assert consts.shape == (P, CONST_COLS)
    mask_row = np.concatenate(
        [mask_d.reshape(-1), mask_a.reshape(-1), mask_e.reshape(-1)]
    ).astype(np.float32)[None, :]
    assert mask_row.shape == (1, MASK_COLS)

    in_maps = []
    for c in range(N_CORES):
        s = slice(c * BC, (c + 1) * BC)
        in_maps.append({
            "distance": np.ascontiguousarray(f["distance"][s], np.float32),
            "azimuth": np.ascontiguousarray(f["azimuth"][s], np.float32),
            "elevation": np.ascontiguousarray(f["elevation"][s], np.float32),
            "spikes": np.ascontiguousarray(
                f["spikes"][s].reshape(BC, EC, T), np.float32),
            "consts": consts,
            "mask_row": np.ascontiguousarray(mask_row, np.float32),
            "m2q": np.ascontiguousarray(m2q_host, np.float32),
            "bias_cat": np.ascontiguousarray(bias_cat, np.float32),
        })
    return in_maps


# ---------------------------------------------------------------- entry
def kernel(**inputs):
    in_maps = host_prep(inputs)
    nc = get_graph()
    res = run_bass_kernel_spmd(nc, in_maps, core_ids=list(range(N_CORES)))
    return np.concatenate([r["out"] for r in res.results], axis=0)


# revision 66
# speedup vs baseline: 1.1677x; 1.1677x over previous
"""Trainium2 Bass kernel for nn_AdaptiveResidualCombinedEncoder.

Pure data-parallel over 8 NeuronCores: batch 2048 -> 256 rows/core.

Key idea: all the soft-shift ops (_shift_last / _shift_channels) have
batch-independent indices & lerp weights, so they are banded linear
operators that the host can precompute:
  - per-branch shift "masks" (band diagonals, gains folded in) applied
    with shifted-AP vector ops,
  - a [128,128] block-diagonal channel-shift matrix for the spikes
    branch applied as bf16 TensorEngine matmuls (two k=64 chunks per
    row so the HBM reads use 2KB descriptors via channel-pair packing),
  - split base/residual weight stacks so relu(base) + scale*res uses
    two accumulated matmul groups (bias via a k=1 ones matmul).
The spikes t-summary falls out of the PSUM->SBUF evacuation for free via
the accum_out feature (per-row running sum on DVE/ACT), and the whole
kernel is HBM-bandwidth-bound (~74MB/core) at ~92% of the practical
packet ceiling.
"""

from contextlib import ExitStack

import numpy as np

import concourse.bass as bass
import concourse.tile as tile
from concourse import bacc, mybir
from concourse.bass_utils import run_bass_kernel_spmd

F32 = mybir.dt.float32
BF16 = mybir.dt.bfloat16
ALU = mybir.AluOpType
ACT = mybir.ActivationFunctionType

N_CORES = 8
B = 2048
BC = B // N_CORES            # 256 rows per core
P = 128                      # partitions / rows per tile
NT = BC // P                 # 2 row-tiles per core
EARS, NFC, T = 2, 64, 256
EC = EARS * NFC              # 128 (ears*channels)
D_DIM, A_DIM, E_DIM, H = 256, 256, 192, 512
OUT_COLS = 3 * H + EC * T    # 34304
RG = 8                       # spikes rows per DMA batch
EPS = 1e-5


# ---------------------------------------------------------------- host math
def _np_gain(p):
    return (1.0 + 0.35 * np.tanh(p.astype(np.float64))).astype(np.float32)


def _shift_weights(d, offsets, max_shift):
    """Per-position (lo, hi, a) of the reference soft shift."""
    base = np.arange(d, dtype=np.float32)
    s = base + np.float32(max_shift) * np.tanh(offsets.astype(np.float32))
    s = np.clip(s, 0.0, np.float32(d - 1)).astype(np.float32)
    lo = np.floor(s).astype(np.int64)
    hi = np.ceil(s).astype(np.int64)
    a = (s - lo.astype(np.float32)).astype(np.float32)
    return lo, hi, a


def _band_masks(widths, offs, max_shifts, gains, max_delta):
    """Band-diagonal masks for concatenated shift blocks.

    Returns [2*max_delta+1, sum(widths)] f32: M[di][c] multiplies
    x[c + (di - max_delta)] accumulated into output position c.
    """
    total = int(np.sum(widths))
    n_d = 2 * max_delta + 1
    M = np.zeros((n_d, total), np.float32)
    c0 = 0
    for w, off, ms, g in zip(widths, offs, max_shifts, gains):
        if off is None:  # pure diagonal (gain only)
            M[max_delta, c0:c0 + w] += g
        else:
            lo, hi, a = _shift_weights(w, off, ms)
            for c in range(w):
                M[lo[c] - c + max_delta, c0 + c] += (1.0 - a[c]) * g[c]
                M[hi[c] - c + max_delta, c0 + c] += a[c] * g[c]
        c0 += w
    return M


def _shift_matrix(d, offsets, max_shift, gain):
    """Dense [d, d]: out[c] = sum_src M[c, src] * x[src], gain folded."""
    lo, hi, a = _shift_weights(d, offsets, max_shift)
    M = np.zeros((d, d), np.float32)
    idx = np.arange(d)
    np.add.at(M, (idx, lo), (1.0 - a) * gain)
    np.add.at(M, (idx, hi), a * gain)
    return M


# consts column layout, shared between build_graph and kernel
_CW = {}
_off = 0
for _name, _w in [
    ("wcat_d", 4 * H), ("wcat_a", 4 * H), ("wcat_e", 4 * H),
    ("m2t", EC), ("ident", P),
]:
    _CW[_name] = (_off, _w)
    _off += _w
CONST_COLS = _off

# mask row layout (one-partition param, broadcast on-chip)
_MW = {"mask_d": (0, 3 * D_DIM), "mask_a": (3 * D_DIM, 3 * A_DIM),
       "mask_e": (3 * D_DIM + 3 * A_DIM, 5 * E_DIM)}
MASK_COLS = 3 * D_DIM + 3 * A_DIM + 5 * E_DIM  # 2496


# ---------------------------------------------------------------- device IR
def build_graph(debug=False):
    """One SPMD graph, identical on all cores; values arrive via params."""
    nc = bacc.Bacc(None, target_bir_lowering=False)

    dist_e = nc.dram_tensor("distance", [BC, D_DIM], F32, kind="ExternalInput")
    az_e = nc.dram_tensor("azimuth", [BC, A_DIM], F32, kind="ExternalInput")
    elev_e = nc.dram_tensor("elevation", [BC, E_DIM], F32, kind="ExternalInput")
    sp_e = nc.dram_tensor("spikes", [BC, EC, T], F32, kind="ExternalInput")
    consts_e = nc.dram_tensor("consts", [P, CONST_COLS], F32, kind="ExternalInput")
    mask_e = nc.dram_tensor("mask_row", [1, MASK_COLS], F32, kind="ExternalInput")
    bias_e = nc.dram_tensor("bias_cat", [1, 6 * H], F32, kind="ExternalInput")
    out_e = nc.dram_tensor("out", [BC, OUT_COLS], F32, kind="ExternalOutput")
    if debug:
        dbg_ad = nc.dram_tensor("dbg_ad", [P, D_DIM], F32, kind="ExternalOutput")
        dbg_ln = nc.dram_tensor("dbg_ln", [P, D_DIM], F32, kind="ExternalOutput")
        dbg_lhs = nc.dram_tensor("dbg_lhs", [P, 4 * P], F32, kind="ExternalOutput")
        dbg_ps = nc.dram_tensor("dbg_ps", [P, H], F32, kind="ExternalOutput")
        dbg_sta = nc.dram_tensor("dbg_sta", [P, P], F32, kind="ExternalOutput")
        dbg_lns = nc.dram_tensor("dbg_lns", [P, P], F32, kind="ExternalOutput")

    with ExitStack() as ctx:
        tc = ctx.enter_context(tile.TileContext(nc))
        cpool = ctx.enter_context(tc.tile_pool(name="consts", bufs=1))
        inpool = ctx.enter_context(tc.tile_pool(name="inputs", bufs=1))
        sp_in_pool = ctx.enter_context(tc.tile_pool(name="sp_in", bufs=4))
        sp_out_pool = ctx.enter_context(tc.tile_pool(name="sp_out", bufs=4))
        st_pool = ctx.enter_context(tc.tile_pool(name="st", bufs=2))
        work = ctx.enter_context(tc.tile_pool(name="work", bufs=2))
        lhs_pool = ctx.enter_context(tc.tile_pool(name="lhs", bufs=2))
        lat_pool = ctx.enter_context(tc.tile_pool(name="lat", bufs=2))
        stats = ctx.enter_context(tc.tile_pool(name="stats", bufs=4))
        ps_sp = ctx.enter_context(tc.tile_pool(name="ps_sp", bufs=3, space="PSUM"))
        ps_tr = ctx.enter_context(tc.tile_pool(name="ps_tr", bufs=2, space="PSUM"))
        ps_sum = ctx.enter_context(tc.tile_pool(name="ps_sum", bufs=1, space="PSUM"))
        ps_lat = ctx.enter_context(tc.tile_pool(name="ps_lat", bufs=2, space="PSUM"))

        consts = cpool.tile([P, CONST_COLS], F32)
        nc.sync.dma_start(consts[:], consts_e[:])
        biasr = cpool.tile([1, 6 * H], F32)
        nc.sync.dma_start(biasr[:], bias_e[:])
        ones_r = cpool.tile([1, P], F32)
        nc.vector.memset(ones_r[:], 1.0)
        # bf16 spike-shift matrix via SWDGE cast-DMA, split into
        # even/odd source-channel halves duplicated on both partition
        # halves (for the k=64 channel-pair matmuls)
        m2o = _CW["m2t"][0]
        m2t_ev = cpool.tile([P, EC], BF16)
        ev_src = consts_e[:, m2o:m2o + EC].rearrange(
            "(q pr) m -> pr q m", pr=2)[0]          # [64, EC] even rows
        od_src = consts_e[:, m2o:m2o + EC].rearrange(
            "(q pr) m -> pr q m", pr=2)[1]
        nc.gpsimd.dma_start(m2t_ev[0:64, :], ev_src)
        nc.gpsimd.dma_start(m2t_ev[64:P, :], ev_src)
        m2t_od = cpool.tile([P, EC], BF16)
        nc.gpsimd.dma_start(m2t_od[0:64, :], od_src)
        nc.gpsimd.dma_start(m2t_od[64:P, :], od_src)
        eps_t = cpool.tile([P, 1], F32)
        nc.vector.memset(eps_t[:], float(EPS))

        def cslice(name, j=None, w=None):
            o, full = _CW[name]
            if j is not None:
                return consts[:, o + j * w: o + (j + 1) * w]
            return consts[:, o: o + full]

        ident = cslice("ident")

        # broadcast the 1-partition mask row to all 128 partitions via PE
        mrow = cpool.tile([1, MASK_COLS], F32)
        nc.sync.dma_start(mrow[:], mask_e[:])
        masks = cpool.tile([P, MASK_COLS], F32)
        for j in range((MASK_COLS + H - 1) // H):
            c0, c1 = j * H, min((j + 1) * H, MASK_COLS)
            psm = ps_lat.tile([P, H], F32, tag="lat")
            nc.tensor.matmul(psm[:, 0:c1 - c0], ones_r[:], mrow[:, c0:c1],
                             start=True, stop=True)
            nc.vector.tensor_copy(masks[:, c0:c1], psm[:, 0:c1 - c0])

        # whole-core input loads, [P, NT, F] so tile t is [:, t, :]
        distL = inpool.tile([P, NT, D_DIM], F32)
        nc.sync.dma_start(distL[:], dist_e[:].rearrange("(t p) f -> p t f", p=P))
        azL = inpool.tile([P, NT, A_DIM], F32)
        nc.sync.dma_start(azL[:], az_e[:].rearrange("(t p) f -> p t f", p=P))
        elevL = inpool.tile([P, NT, E_DIM], F32)
        nc.sync.dma_start(elevL[:], elev_e[:].rearrange("(t p) f -> p t f", p=P))

        def adapted_from_masks(x, mask_name, width, ndelta):
            """ad[:, c] = sum_d x[:, c + d - md] * M_d[:, c] via shifted APs."""
            md = ndelta // 2
            ad = work.tile([P, width], F32, tag=f"ad_{width}")
            tmp = work.tile([P, width], F32, tag=f"tmp_{width}")
            o, _ = _MW[mask_name]
            mk = lambda j: masks[:, o + j * width: o + (j + 1) * width]
            nc.vector.tensor_tensor(ad[:], x, mk(md), op=ALU.mult)
            for d in range(ndelta):
                sh = d - md  # source offset
                if sh == 0:
                    continue
                if sh < 0:
                    dst, src = slice(-sh, width), slice(0, width + sh)
                else:
                    dst, src = slice(0, width - sh), slice(sh, width)
                nc.vector.tensor_tensor(tmp[:, dst], x[:, src], mk(d)[:, dst],
                                        op=ALU.mult)
                nc.vector.tensor_tensor(ad[:, dst], ad[:, dst], tmp[:, dst],
                                        op=ALU.add)
            return ad

        def layernorm(x_ap, width, tag):
            """Return ln tile [P, width] (SBUF), rows on partitions."""
            st6 = stats.tile([P, 6], F32, tag=f"st6_{tag}")
            nc.vector.bn_stats(st6[:], x_ap)
            mv = stats.tile([P, 2], F32, tag=f"mv_{tag}")
            nc.vector.bn_aggr(mv[:], st6[:])
            std = stats.tile([P, 1], F32, tag=f"std_{tag}")
            nc.scalar.activation(std[:], mv[:, 1:2], ACT.Sqrt, bias=eps_t[:])
            rstd = stats.tile([P, 1], F32, tag=f"rstd_{tag}")
            nc.vector.reciprocal(rstd[:], std[:])
            ln = work.tile([P, width], F32, tag=f"ln_{tag}")
            nc.vector.tensor_scalar(ln[:], x_ap, mv[:, 0:1], rstd[:],
                                    op0=ALU.subtract, op1=ALU.mult)
            return ln

        def transpose_into(dst_ap, src_ap, ncols):
            """dst[f, r] = src[r, f]; src [P, ncols] -> dst [ncols, P]."""
            tr = ps_tr.tile([P, P], F32, tag="tr")
            nc.tensor.transpose(tr[:ncols, :], src_ap, ident)
            nc.vector.tensor_copy(dst_ap, tr[:ncols, :])

        def mm_group(ps, chunks, bias_off, wname):
            """PSUM group: ones@bias then each (lhs_ap, w_ap) chunk."""
            nc.tensor.matmul(ps[:], ones_r[:],
                             biasr[:, bias_off:bias_off + H],
                             start=True, stop=False)
            for i, (lhs_ap, w_ap) in enumerate(chunks):
                nc.tensor.matmul(ps[:], lhs_ap, w_ap,
                                 start=False, stop=(i == len(chunks) - 1))

        def branch_epilogue(ps_base, ps_res, boff, t):
            """lat = relu(relu(base) + res) -> out."""
            rb = lat_pool.tile([P, H], F32, tag="relu_base")
            nc.scalar.activation(rb[:], ps_base[:], ACT.Relu)
            pre = lat_pool.tile([P, H], F32, tag="lat_pre")
            nc.vector.scalar_tensor_tensor(pre[:], ps_res[:], 1.0, rb[:],
                                           op0=ALU.mult, op1=ALU.add)
            lat = lat_pool.tile([P, H], F32, tag="lat_sb")
            nc.scalar.activation(lat[:], pre[:], ACT.Relu)
            nc.sync.dma_start(out_e[t * P:(t + 1) * P, boff:boff + H], lat[:])

        def spikes_batch(STa, r0, g):
            b0 = r0 + g * RG
            # channel-pair layout: partition q<64 holds chans (2q, 2q+1)
            # of even rows, q>=64 of odd rows -> 2KB HBM read descriptors
            spi = sp_in_pool.tile([P, RG // 2, 2 * T], BF16, tag="spi")
            src = sp_e[b0:b0 + RG].rearrange(
                "(r2 two) (q pr) t -> two q r2 (pr t)", two=2, pr=2)
            nc.gpsimd.dma_start(spi[0:64, :, :], src[0])
            nc.gpsimd.dma_start(spi[64:P, :, :], src[1])
            spo = sp_out_pool.tile([P, RG, T], F32, tag="spo")
            for r in range(RG):
                r2, base = r // 2, (r % 2) * 64
                psb = ps_sp.tile([P, T], F32, tag="ps_sp")
                nc.tensor.matmul(
                    psb[:], m2t_ev[base:base + 64, :],
                    spi[base:base + 64, r2, 0:T], start=True, stop=False)
                nc.tensor.matmul(
                    psb[:], m2t_od[base:base + 64, :],
                    spi[base:base + 64, r2, T:2 * T], start=False, stop=True)
                c0 = g * RG + r
                if (r // 2) % 2 == 0:   # DVE: copy + per-row sum
                    nc.vector.tensor_scalar(
                        spo[:, r, :], psb[:], 1.0, None,
                        op0=ALU.mult, op1=ALU.add,
                        accum_out=STa[:, c0:c0 + 1])
                else:                   # ACT: copy + per-row sum
                    nc.scalar.activation(
                        spo[:, r, :], psb[:], ACT.Copy,
                        accum_out=STa[:, c0:c0 + 1])
            nc.sync.dma_start(
                out_e[b0:b0 + RG, 3 * H:].rearrange("r (p t) -> p r t", p=P),
                spo[:])

        def branch_d(t):
            # -------- distance branch
            xd = distL[:, t, :]
            ad = adapted_from_masks(xd, "mask_d", D_DIM, 3)
            ln_d = layernorm(ad[:], D_DIM, "d")
            lhs_d = lhs_pool.tile([P, 4, P], F32, tag="lhs_d")
            transpose_into(lhs_d[:, 0, :], xd[:, 0:P], P)
            transpose_into(lhs_d[:, 1, :], xd[:, P:2 * P], P)
            transpose_into(lhs_d[:, 2, :], ln_d[:, 0:P], P)
            transpose_into(lhs_d[:, 3, :], ln_d[:, P:2 * P], P)
            if debug and t == 0:
                nc.sync.dma_start(dbg_ad[:], ad[:])
                nc.sync.dma_start(dbg_ln[:], ln_d[:])
                nc.sync.dma_start(
                    dbg_lhs[:], lhs_d[:].rearrange("p a b -> p (a b)"))
            ps_b = ps_lat.tile([P, H], F32, tag="lat")
            mm_group(ps_b, [(lhs_d[:, j, :], cslice("wcat_d", j, H))
                            for j in (0, 1)], 0 * H, "wcat_d")
            ps_r = ps_lat.tile([P, H], F32, tag="lat")
            mm_group(ps_r, [(lhs_d[:, j, :], cslice("wcat_d", j, H))
                            for j in (2, 3)], 3 * H, "wcat_d")
            branch_epilogue(ps_b, ps_r, 0 * H, t)

        def branch_a(t):
            xa = azL[:, t, :]
            aa = adapted_from_masks(xa, "mask_a", A_DIM, 3)
            ln_a = layernorm(aa[:], A_DIM, "a")
            lhs_a = lhs_pool.tile([P, 4, P], F32, tag="lhs_a")
            transpose_into(lhs_a[:, 0, :], xa[:, 0:P], P)
            transpose_into(lhs_a[:, 1, :], xa[:, P:2 * P], P)
            transpose_into(lhs_a[:, 2, :], ln_a[:, 0:P], P)
            transpose_into(lhs_a[:, 3, :], ln_a[:, P:2 * P], P)
            ps_b = ps_lat.tile([P, H], F32, tag="lat")
            mm_group(ps_b, [(lhs_a[:, j, :], cslice("wcat_a", j, H))
                            for j in (0, 1)], 1 * H, "wcat_a")
            ps_r = ps_lat.tile([P, H], F32, tag="lat")
            mm_group(ps_r, [(lhs_a[:, j, :], cslice("wcat_a", j, H))
                            for j in (2, 3)], 4 * H, "wcat_a")
            branch_epilogue(ps_b, ps_r, 1 * H, t)

        def branch_e(STa, t):
            # -------- elevation (+ spike-summary residual) branch
            xe = elevL[:, t, :]
            ae = adapted_from_masks(xe, "mask_e", E_DIM, 5)
            ln_e = layernorm(ae[:], E_DIM, "e")

            # summary rows: transpose STa -> [rows, EC feats], LN from PSUM
            trs = ps_sum.tile([P, P], F32, tag="trs")
            nc.tensor.transpose(trs[:], STa[:], ident)
            ln_s = layernorm(trs[:], EC, "s")
            if debug and t == 0:
                nc.sync.dma_start(dbg_sta[:], STa[:])
                nc.sync.dma_start(dbg_lns[:], ln_s[:])

            lhs_e = lhs_pool.tile([P, 4, P], F32, tag="lhs_e")
            transpose_into(lhs_e[:, 0, :], xe[:, 0:P], P)
            # chunk 1 mixes elev[128:192] and ln_e[0:64]: concat in free
            # dim first, then one full 128-wide transpose
            ecat = work.tile([P, P], F32, tag="ecat")
            nc.vector.tensor_copy(ecat[:, 0:64], xe[:, P:E_DIM])
            nc.vector.tensor_copy(ecat[:, 64:P], ln_e[:, 0:64])
            transpose_into(lhs_e[:, 1, :], ecat[:], P)
            transpose_into(lhs_e[:, 2, :], ln_e[:, 64:E_DIM], P)
            transpose_into(lhs_e[:, 3, :], ln_s[:], P)
            ps_b = ps_lat.tile([P, H], F32, tag="lat")
            mm_group(ps_b, [
                (lhs_e[:, 0, :], cslice("wcat_e", 0, H)),
                (lhs_e[0:64, 1, :], cslice("wcat_e", 1, H)[0:64, :]),
            ], 2 * H, "wcat_e")
            ps_r = ps_lat.tile([P, H], F32, tag="lat")
            mm_group(ps_r, [
                (lhs_e[64:P, 1, :], cslice("wcat_e", 1, H)[64:P, :]),
                (lhs_e[:, 2, :], cslice("wcat_e", 2, H)),
                (lhs_e[:, 3, :], cslice("wcat_e", 3, H)),
            ], 5 * H, "wcat_e")
            branch_epilogue(ps_b, ps_r, 2 * H, t)

        NB = P // RG
        for t in range(NT):
            r0 = t * P
            STa = st_pool.tile([P, P], F32, tag="STa")
            for g in range(NB):
                spikes_batch(STa, r0, g)
            branch_d(t)
            branch_a(t)
            branch_e(STa, t)

    return nc


_GRAPH_CACHE = {}


def get_graph():
    if "nc" not in _GRAPH_CACHE:
        nc = build_graph()
        nc.finalize()
        _GRAPH_CACHE["nc"] = nc
    return _GRAPH_CACHE["nc"]


def host_prep(inputs):
    """Shard + precompute the derived constant tensors -> in_maps."""
    f = {k: np.asarray(v) for k, v in inputs.items()}
    dh, ah = D_DIM // 2, A_DIM // 2

    mask_d = _band_masks(
        [dh, dh], [f["d_left_off"], f["d_right_off"]], [0.75, 0.75],
        [_np_gain(f["d_left_g"]), _np_gain(f["d_right_g"])], 1)
    mask_a = _band_masks(
        [ah, ah], [f["az_itd_off"], None], [0.75, None],
        [_np_gain(f["az_itd_g"]), _np_gain(f["az_ild_g"])], 1)
    mask_e = _band_masks(
        [NFC, NFC, NFC],
        [f["el_norm_off"], f["el_notch_off"], f["el_slope_off"]],
        [1.5, 1.5, 1.5],
        [_np_gain(f["el_norm_g"]), _np_gain(f["el_notch_g"]),
         _np_gain(f["el_slope_g"])], 2)

    def sigmoid(x):
        return np.float32(1.0 / (1.0 + np.exp(-np.float64(x))))

    d_scale = np.float32(0.35) * sigmoid(f["dist_gain"])
    a_scale = np.float32(0.35) * sigmoid(f["az_gain"])
    e_scale = np.float32(0.35) * sigmoid(f["el_gain"])

    wcat_d = np.vstack([f["bWd"], d_scale * f["Wd"]]).astype(np.float32)
    wcat_a = np.vstack([f["bWa"], a_scale * f["Wa"]]).astype(np.float32)
    wcat_e = np.vstack([f["bWe"], e_scale * f["We"],
                        np.float32(0.25) * e_scale * f["Wsp"]]
                       ).astype(np.float32)
    bias_cat = np.concatenate([
        f["bbd"], f["bba"], f["bbe"],
        d_scale * f["bd"], a_scale * f["ba"],
        e_scale * f["be"] + np.float32(0.25) * e_scale * f["bsp"],
    ]).astype(np.float32)[None, :]

    m_ch = _shift_matrix(NFC, f["spec_off"], 1.5, _np_gain(f["spec_g"]))
    m2 = np.kron(np.eye(EARS, dtype=np.float32), m_ch)   # [EC, EC]
    m2t = np.ascontiguousarray(m2.T)

    consts = np.concatenate([
        wcat_d.reshape(4, P, H).transpose(1, 0, 2).reshape(P, 4 * H),
        wcat_a.reshape(4, P, H).transpose(1, 0, 2).reshape(P, 4 * H),
        wcat_e.reshape(4, P, H).transpose(1, 0, 2).reshape(P, 4 * H),
        m2t, np.eye(P, dtype=np.float32),
    ], axis=1).astype(np.float32)
    consts = np.ascontiguousarray(consts)
    assert consts.shape == (P, CONST_COLS)
    mask_row = np.concatenate(
        [mask_d.reshape(-1), mask_a.reshape(-1), mask_e.reshape(-1)]
    ).astype(np.float32)[None, :]
    assert mask_row.shape == (1, MASK_COLS)

    in_maps = []
    for c in range(N_CORES):
        s = slice(c * BC, (c + 1) * BC)
        in_maps.append({
            "distance": np.ascontiguousarray(f["distance"][s], np.float32),
            "azimuth": np.ascontiguousarray(f["azimuth"][s], np.float32),
            "elevation": np.ascontiguousarray(f["elevation"][s], np.float32),
            "spikes": np.ascontiguousarray(
                f["spikes"][s].reshape(BC, EC, T), np.float32),
            "consts": consts,
            "mask_row": np.ascontiguousarray(mask_row, np.float32),
            "bias_cat": np.ascontiguousarray(bias_cat, np.float32),
        })
    return in_maps


# ---------------------------------------------------------------- entry
def kernel(**inputs):
    in_maps = host_prep(inputs)
    nc = get_graph()
    res = run_bass_kernel_spmd(nc, in_maps, core_ids=list(range(N_CORES)))
    return np.concatenate([r["out"] for r in res.results], axis=0)
